# revision 27
# baseline (speedup 1.0000x reference)
"""HGT link predictor on 8 Trainium2 NeuronCores (Bass/Tile SPMD kernel).

Strategy (hardcoded for nn_HGTLinkPredictor, N=50000 E=800000 P=100000 C=128 H=4 D=32):
 - Shard dst nodes (and their incoming edges) across 8 cores in contiguous
   128-node blocks; edges sorted by dst on host.
 - Features flow in fp16. Node features are kept TRANSPOSED ([C, n]) in SBUF
   so q/k/v projections are a single 384-wide matmul per 128-node block with
   no on-device transposes; relation transforms + attention scale are folded
   into the weights on host.
 - k and v rows are concatenated ([N, 256] fp16); per-edge rows are fetched
   with gpsimd.dma_gather (up to 2048 int16 indices per instruction — the
   994ns fixed SWDGE overhead dominates, so calls are as large as possible).
   The kv table is split into lo/hi halves so row indices fit int16; the lo
   half is the concat of each core's FIRST 25 blocks (hi: last 24), so the
   kv AllGather runs as two chunked collectives: AG#lo starts while the
   second half of the projections still runs, and the lo-half edge gathers
   proceed while AG#hi is still in flight. Trailing pad slots use idx=-1
   (SWDGE skips them); gather buffers are memset once at start so skipped
   rows read finite stale data.
 - q is never round-tripped through DRAM or gathered: q rows stay in SBUF
   ([n, c] per block) and per-edge q is expanded on the tensor engine with
   host-precomputed one-hot selection matrices (qg = ST_t.T @ q_blk), into
   fp16 PSUM so the k*q multiply runs in the DVE 2x perf mode.
 - Segment softmax/weighted-sum per 128-node block via the same one-hot
   matrices on the PE (shipped as fp8, 0/1 exact, vs fp16 rhs); the
   denominator rides along as 4 extra rhs columns and division is deferred
   to the block epilogue. exp() is emitted on the scalar engine directly in
   the 32-wide-replicated layout the v*ex multiply needs (so that multiply
   also runs in DVE 2x mode). alpha stays well under ln(65504) here.
 - gelu uses the tanh approximation (Square/Tanh/Identity/Exp all live in
   activation table set 0, so the scalar engine loads ONE table total; the
   exact-gelu table thrashed 114 reloads x 1.3us). The 0.5 gelu factor is
   folded into Wo on the host.
 - Epilogue is done transposed (lhsT=Wo trick) so h1^T stays in SBUF for
   layer 2 and the link decode is a [C,2]-stationary matmul per block.
"""

import math
import os
import numpy as np
from contextlib import ExitStack

import concourse.bass as bass
import concourse.tile as tile
from concourse import bacc, mybir
from concourse import bass_utils
from concourse.masks import make_identity
from concourse import library_config

F32 = mybir.dt.float32
F16 = mybir.dt.float16
I16 = mybir.dt.int16
F8 = mybir.dt.float8e4
AF = mybir.ActivationFunctionType
OP = mybir.AluOpType

CORES = 8
EPS = 1e-30
GCHUNK = int(os.environ.get("HGT_GCHUNK", "8"))   # dma_gather tiles per instruction (1024-idx HW cap)
GBLK = int(os.environ.get("HGT_GBLK", "3"))       # blocks per gather group
EX32 = os.environ.get("HGT_EX32", "1") == "1"
CCHUNK = 8          # compute (qg/kq) tiles per chunk
GELU_C = 0.044715
GELU_S = 0.7978845608028654


def _v(ap, off, dims):
    """Custom free-dim view of a 2D [part, width] AP: keep partition dim,
    replace free dims with `dims` ([step, num] pairs), add `off` elements."""
    return bass.AP(ap.tensor, ap.offset + off, [list(ap.ap[0])] + [list(d) for d in dims])


def _wrap16(flat):
    """[M*16] -> [16, M] with element i at [i%16, i//16]."""
    return flat.reshape(-1, 16).T.copy()


# ----------------------------------------------------------------- host prep

def _host_prep(x, edge_index, pos_edge_index, neg_edge_index):
    N, C = x.shape
    E = edge_index.shape[1]
    P = pos_edge_index.shape[1]

    NPC = int(math.ceil(N / (CORES * 128))) * 128   # nodes per core (padded)
    BPC = NPC // 128                                # blocks per core
    NPAD = NPC * CORES
    LOB = (BPC + 1) // 2                            # lo-half blocks per core
    HIB = BPC - LOB
    LO_NPC = LOB * 128                              # lo-half nodes per core
    HI_NPC = HIB * 128
    LO = LO_NPC * CORES                             # lo kv table rows
    HI = HI_NPC * CORES
    assert LO < 2 ** 15 and HI < 2 ** 15

    HALF = NPAD // 2
    src = edge_index[0].astype(np.int64)
    dst = edge_index[1].astype(np.int64)
    order = np.argsort(dst, kind="stable")
    s_src, s_dst = src[order], dst[order]

    core_of = s_dst // NPC
    blk_of = (s_dst % NPC) // 128
    gblk = core_of * BPC + blk_of
    ishigh = (s_src >= HALF).astype(np.int64)
    kvrow = s_src - ishigh * HALF

    # reorder within each (core, block): lo-half src first
    order2 = np.argsort(gblk * 2 + ishigh, kind="stable")
    s_src, s_dst = s_src[order2], s_dst[order2]
    core_of, blk_of, gblk, ishigh, kvrow = (core_of[order2], blk_of[order2],
                                            gblk[order2], ishigh[order2],
                                            kvrow[order2])

    # per (core, block, half) counts -> shared tile counts per block index
    cnt = np.zeros((CORES, BPC, 2), dtype=np.int64)
    np.add.at(cnt, (core_of, blk_of, ishigh), 1)
    T1_b = np.ceil(cnt[:, :, 0].max(axis=0) / 128).astype(np.int64)  # [BPC]
    T2_b = np.ceil(cnt[:, :, 1].max(axis=0) / 128).astype(np.int64)
    empty = (T1_b + T2_b) == 0
    T1_b[empty] = 1
    T_b = T1_b + T2_b
    tiles_total = int(T_b.sum())

    # gather-group layout: blocks are grouped GBLK at a time; within a group
    # all T1 tile sections are contiguous, then all T2 sections, so one
    # dma_gather call spans block boundaries (fewer 994ns fixed overheads).
    ngrp = (BPC + GBLK - 1) // GBLK
    t1_start = np.zeros(BPC, dtype=np.int64)   # flat tile index of block's T1
    t2_start = np.zeros(BPC, dtype=np.int64)
    g_t1 = np.zeros(ngrp, dtype=np.int64)      # group T1 section start / len
    g_t1n = np.zeros(ngrp, dtype=np.int64)
    g_t2 = np.zeros(ngrp, dtype=np.int64)
    g_t2n = np.zeros(ngrp, dtype=np.int64)
    pos = 0
    for g in range(ngrp):
        bs = range(g * GBLK, min((g + 1) * GBLK, BPC))
        g_t1[g] = pos
        for b in bs:
            t1_start[b] = pos
            pos += T1_b[b]
        g_t1n[g] = pos - g_t1[g]
        g_t2[g] = pos
        for b in bs:
            t2_start[b] = pos
            pos += T2_b[b]
        g_t2n[g] = pos - g_t2[g]
    assert pos == tiles_total

    # rank of each edge within its (core, block, half) group
    ghalf = gblk * 2 + ishigh
    grp_start = np.zeros(CORES * BPC * 2 + 1, dtype=np.int64)
    np.add.at(grp_start, ghalf + 1, 1)
    grp_start = np.cumsum(grp_start)
    pos_in_grp = np.arange(E) - grp_start[ghalf]

    # flat slot within the core's [tiles_total*128] edge array
    flat_pos = (np.where(ishigh == 0, t1_start[blk_of], t2_start[blk_of]) * 128
                + pos_in_grp)

    cap = tiles_total * 128
    PADIDX = int(os.environ.get("HGT_PADIDX", "0"))
    kvidx = np.full((CORES, cap), PADIDX, dtype=np.int16)
    eslot = np.full((CORES, cap), -1, dtype=np.int64)

    kvidx[core_of, flat_pos] = kvrow.astype(np.int16)
    eslot[core_of, flat_pos] = s_dst % 128

    import ml_dtypes
    # one-hot selection matrices, [128, tiles_total*128] fp8 (0/1 exact)
    #   S[p, t*128 + n]  = (eslot[edge t*128+p] == n)
    #   ST[n, t*128 + p] = (eslot[edge t*128+p] == n)
    S = np.zeros((CORES, 128, tiles_total * 128), dtype=ml_dtypes.float8_e4m3)
    ST = np.zeros((CORES, 128, tiles_total * 128), dtype=ml_dtypes.float8_e4m3)
    for c in range(CORES):
        i = np.arange(cap)
        valid = eslot[c] >= 0
        iv, sl = i[valid], eslot[c][valid]
        S[c, iv % 128, (iv // 128) * 128 + sl] = 1.0
        ST[c, sl, iv] = 1.0

    kv16 = np.zeros((CORES, 128, tiles_total * 8), dtype=np.int16)
    for c in range(CORES):
        # the SWDGE ucode reads the [16, M] wrap from partition group
        # 2*queue_num(+1); replicate everywhere so any queue works
        kv16[c] = np.tile(_wrap16(kvidx[c]), (8, 1))

    # x shards, transposed: [C, NPC] fp16 (plus residual-prescaled copy)
    xpad = np.zeros((NPAD, C), dtype=np.float32)
    xpad[:N] = x
    xT = np.zeros((CORES, C, NPC), dtype=np.float16)
    for c in range(CORES):
        xT[c] = xpad[c * NPC:(c + 1) * NPC].T.astype(np.float16)

    meta = dict(N=N, C=C, E=E, P=P, NPC=NPC, BPC=BPC, NPAD=NPAD,
                LOB=LOB, HIB=HIB, LO=LO, HI=HI,
                T1_b=tuple(int(t) for t in T1_b),
                T2_b=tuple(int(t) for t in T2_b),
                t1_start=tuple(int(t) for t in t1_start),
                t2_start=tuple(int(t) for t in t2_start),
                g_t1=tuple(int(t) for t in g_t1),
                g_t1n=tuple(int(t) for t in g_t1n),
                g_t2=tuple(int(t) for t in g_t2),
                g_t2n=tuple(int(t) for t in g_t2n),
                tiles_total=tiles_total)
    arrays = dict(kv16=kv16, S=S, ST=ST, xT=xT,
                  ident=np.eye(128, dtype=np.float16))
    return meta, arrays


def _prep_weights(inputs, H, D):
    """Fold relation transforms + attention scale into the linear weights."""
    C = inputs["W1k"].shape[0]
    out = {}
    for l in (1, 2):
        a_rel = np.asarray(inputs[f"a{l}"], np.float64)
        m_rel = np.asarray(inputs[f"m{l}"], np.float64)
        p_rel = np.asarray(inputs[f"p{l}"], np.float64)
        A = np.zeros((C, C)); M = np.zeros((C, C))
        for h in range(H):
            A[h * D:(h + 1) * D, h * D:(h + 1) * D] = a_rel[h]
            M[h * D:(h + 1) * D, h * D:(h + 1) * D] = m_rel[h]
        qscale = np.repeat(p_rel / np.sqrt(D), D)
        Wq = np.asarray(inputs[f"W{l}q"], np.float64) * qscale
        bq = np.asarray(inputs[f"b{l}q"], np.float64) * qscale
        Wk = np.asarray(inputs[f"W{l}k"], np.float64) @ A
        bk = np.asarray(inputs[f"b{l}k"], np.float64) @ A
        Wv = np.asarray(inputs[f"W{l}v"], np.float64) @ M
        bv = np.asarray(inputs[f"b{l}v"], np.float64) @ M
        a_sig = float(1.0 / (1.0 + np.exp(-float(inputs[f"skip{l}"]))))
        Wqkv = np.concatenate([Wq, Wk, Wv], axis=1)        # [C, 384]
        bqkv = np.concatenate([bq, bk, bv])                # [384]
        out[f"Wqkv{l}"] = Wqkv.astype(np.float16)
        out[f"bqkv{l}"] = np.broadcast_to(bqkv.astype(np.float32), (128, 3 * C)).copy()
        # 0.5 of the tanh-approx gelu is folded into Wo
        out[f"Wo{l}"] = (0.5 * np.asarray(inputs[f"Wo{l}"], np.float64)).astype(np.float16)
        out[f"boaT{l}"] = (a_sig * np.asarray(inputs[f"bo{l}"], np.float64)
                           ).astype(np.float32).reshape(C, 1).copy()
        out[f"asig{l}"] = a_sig
    Wlp = np.asarray(inputs["Wlp"], np.float32)
    out["w12"] = np.stack([Wlp[:C, 0], Wlp[C:, 0]], axis=1).astype(np.float16)  # [C,2]
    out["w12b"] = ((1.0 - out["asig2"]) * np.stack([Wlp[:C, 0], Wlp[C:, 0]], axis=1)
                   ).astype(np.float16)
    out["blp"] = float(np.asarray(inputs["blp"]).reshape(-1)[0])
    return out


# ------------------------------------------------------------------- program

def _build_program(meta, asig1, asig2, gelu_mode="hw", nqueues=4):
    NPC, BPC, NPAD = meta["NPC"], meta["BPC"], meta["NPAD"]
    LOB, HIB, LO, HI = meta["LOB"], meta["HIB"], meta["LO"], meta["HI"]
    LO_NPC = LOB * 128
    T1_b, T2_b = meta["T1_b"], meta["T2_b"]
    t1_start, t2_start = meta["t1_start"], meta["t2_start"]
    g_t1, g_t1n = meta["g_t1"], meta["g_t1n"]
    g_t2, g_t2n = meta["g_t2"], meta["g_t2n"]
    ngrp = len(g_t1)
    T1G = max(g_t1n)
    T2G = max(max(g_t2n), 1)
    tiles_total = meta["tiles_total"]
    T_b = [a + b for a, b in zip(T1_b, T2_b)]
    Tmax = max(T_b)
    C = meta["C"]

    nc = bacc.Bacc("TRN2", target_bir_lowering=False, debug=False,
                   num_devices=CORES, num_swdge_queues=nqueues)

    # --- I/O -------------------------------------------------------------
    xT_in = nc.dram_tensor("xT", [C, NPC], F16, kind="ExternalInput").ap()
    id_in = nc.dram_tensor("ident_in", [128, 128], F16, kind="ExternalInput").ap()
    kv16_in = nc.dram_tensor("kv16", [128, tiles_total * 8], I16,
                             kind="ExternalInput").ap()
    S_in = nc.dram_tensor("S_hot", [128, tiles_total * 128], F8,
                          kind="ExternalInput").ap()
    ST_in = nc.dram_tensor("ST_hot", [128, tiles_total * 128], F8,
                           kind="ExternalInput").ap()
    w_specs = [("Wqkv1", [C, 3 * C], F16), ("Wqkv2", [C, 3 * C], F16),
               ("bqkv1", [128, 3 * C], F32), ("bqkv2", [128, 3 * C], F32),
               ("Wo1", [C, C], F16), ("Wo2", [C, C], F16),
               ("boaT1", [C, 1], F32), ("boaT2", [C, 1], F32),
               ("w12", [C, 2], F16), ("w12b", [C, 2], F16)]
    w_in = {n: nc.dram_tensor(n, shp, dt, kind="ExternalInput").ap()
            for (n, shp, dt) in w_specs}
    uv_out = nc.dram_tensor("uvT_out", [2, NPC], F32, kind="ExternalOutput").ap()

    with tile.TileContext(nc) as tc, ExitStack() as ctx:
        sb = ctx.enter_context(tc.tile_pool(name="sb", bufs=4))
        sbs = ctx.enter_context(tc.tile_pool(name="sbs", bufs=4))
        cpool = ctx.enter_context(tc.tile_pool(name="const", bufs=1))
        psA = ctx.enter_context(tc.tile_pool(name="psA", bufs=1, space="PSUM"))
        psG = ctx.enter_context(tc.tile_pool(name="psG", bufs=2, space="PSUM"))
        psQ = ctx.enter_context(tc.tile_pool(name="psQ", bufs=1, space="PSUM"))
        psB = ctx.enter_context(tc.tile_pool(name="psB", bufs=1, space="PSUM"))
        expool = ctx.enter_context(tc.tile_pool(name="expool", bufs=2))
        gpool = ctx.enter_context(tc.tile_pool(name="gpool", bufs=2))
        dram = ctx.enter_context(tc.tile_pool(name="dr", bufs=1, space="DRAM"))

        # --- constants into SBUF ----------------------------------------
        W = {}
        for (n, shp, dt) in w_specs:
            W[n] = cpool.tile(shp, dt, tag=f"w_{n}", name=f"wt_{n}")
            nc.sync.dma_start(W[n][:], w_in[n][:])
        kv16_sb = cpool.tile([128, tiles_total * 8], I16, tag="kv16")
        nc.sync.dma_start(kv16_sb[:], kv16_in[:])
        xT_sb = cpool.tile([C, NPC], F16, tag="xT")
        nc.sync.dma_start(xT_sb[:], xT_in[:])

        ident = cpool.tile([128, 128], F16, tag="ident")
        nc.sync.dma_start(ident[:], id_in[:])
        # scaled identity for the layer-1 skip: h1 = a*(Wo@g + bo) + (1-a)*x
        # folds into the output-proj matmul as += ((1-a)/a)*I @ xT.
        skipid = cpool.tile([128, 128], F16, tag="skipid")
        nc.scalar.activation(skipid[:], ident[:], AF.Identity,
                             scale=(1.0 - asig1) / asig1)
        # dma_gather lives in the 'mlp' GPSIMD ucode library
        nc.gpsimd.load_library(library_config.mlp)

        h1T_lo = cpool.tile([C, LO_NPC], F16, tag="h1T_lo")
        h1T_hi = cpool.tile([C, NPC - LO_NPC], F16, tag="h1T_hi")
        qall = cpool.tile([128, BPC * C], F16, tag="qall")
        aggn_all = cpool.tile([128, BPC * 128], F16, tag="aggn_all")
        HALF = NPAD // 2

        # --- DRAM scratch ------------------------------------------------
        kv_shard = dram.tile([NPC, 2 * C], F16, tag="kvs", name="kv_shard")
        kv_full = [dram.tile([NPAD, 2 * C], F16, tag=f"kvf{l}", name=f"kv_full{l}",
                             addr_space="Shared") for l in (0, 1)]

        def srcT_of(layer_idx, b):
            """Transposed features for block b (layer 1: x, layer 2: h1)."""
            if layer_idx == 0:
                return xT_sb[:, b * 128:(b + 1) * 128]
            if b < LOB:
                return h1T_lo[:, b * 128:(b + 1) * 128]
            return h1T_hi[:, (b - LOB) * 128:(b - LOB + 1) * 128]

        def layer(li, asig):
            l = li + 1
            kvf = kv_full[li]
            kvlo, kvhi = kvf[0:HALF, :], kvf[HALF:NPAD, :]
            # ---- projections: one matmul per block ----
            for b in range(BPC):
                blk = slice(b * 128, (b + 1) * 128)
                ps = psA.tile([128, 3 * C], F32, tag="proj")
                nc.tensor.matmul(out=ps[:], lhsT=srcT_of(li, b), rhs=W[f"Wqkv{l}"][:],
                                 start=True, stop=True)
                nc.vector.tensor_tensor(out=qall[:, blk], in0=ps[:, 0:C],
                                        in1=W[f"bqkv{l}"][:, 0:C], op=OP.add)
                qkv = sb.tile([128, 2 * C], F16, tag="qkv")
                nc.vector.tensor_tensor(out=qkv[:], in0=ps[:, C:3 * C],
                                        in1=W[f"bqkv{l}"][:, C:3 * C], op=OP.add)
                nc.sync.dma_start(kv_shard[blk, :], qkv[:])
            nc.gpsimd.collective_compute(
                "AllGather", OP.bypass,
                replica_groups=[list(range(CORES))],
                ins=[kv_shard[:]], outs=[kvf[:]])

            # ---- edge pass A: gather + attention + aggregate ----
            gq = [0]

            def gather_rows(dst, dst_off, table, col8, ntiles):
                done = 0
                while done < ntiles:
                    k = min(GCHUNK, ntiles - done)
                    nc.gpsimd.dma_gather(
                        out_ap=_v(dst[:], dst_off + done * 256,
                                  [[256, k], [1, 256]]),
                        in_ap=table,
                        idxs_ap=kv16_sb[:, (col8 + done) * 8:(col8 + done + k) * 8],
                        num_idxs=k * 128, num_idxs_reg=k * 128,
                        elem_size=256, queue_num=gq[0] % nqueues)
                    gq[0] += 1
                    done += k

            for g in range(ngrp):
                # one pair of gather buffers per GBLK-block group; gather
                # calls span block boundaries (amortizes the 994ns SWDGE
                # fixed overhead across blocks).
                kvg1 = gpool.tile([128, T1G * 256], F16, tag="kvg1")
                kvg2 = gpool.tile([128, T2G * 256], F16, tag="kvg2")
                if g_t1n[g]:
                    gather_rows(kvg1, 0, kvlo, g_t1[g], g_t1n[g])
                if g_t2n[g]:
                    gather_rows(kvg2, 0, kvhi, g_t2[g], g_t2n[g])
                for b in range(g * GBLK, min((g + 1) * GBLK, BPC)):
                    T1, T2 = T1_b[b], T2_b[b]
                    T = T1 + T2
                    blk = slice(b * 128, (b + 1) * 128)
                    o1 = t1_start[b] - g_t1[g]   # block's tile offset in kvg1
                    o2 = t2_start[b] - g_t2[g]
                    S = sb.tile([128, Tmax * 128], F8, tag="S")
                    if T1:
                        nc.sync.dma_start(
                            S[:, :T1 * 128],
                            S_in[:, t1_start[b] * 128:(t1_start[b] + T1) * 128])
                    if T2:
                        nc.sync.dma_start(
                            S[:, T1 * 128:T * 128],
                            S_in[:, t2_start[b] * 128:(t2_start[b] + T2) * 128])
                    ST = sb.tile([128, Tmax * 128], F8, tag="ST")
                    if T1:
                        nc.sync.dma_start(
                            ST[:, :T1 * 128],
                            ST_in[:, t1_start[b] * 128:(t1_start[b] + T1) * 128])
                    if T2:
                        nc.sync.dma_start(
                            ST[:, T1 * 128:T * 128],
                            ST_in[:, t2_start[b] * 128:(t2_start[b] + T2) * 128])
                    kq = sb.tile([128, Tmax * 128], F16, tag="kq")
                    halves = ((kvg1, 0, o1, T1), (kvg2, T1, o2, T2))
                    for (kvg, toff, goff, tn) in halves:
                        for c0 in range(0, tn, CCHUNK):
                            k = min(CCHUNK, tn - c0)
                            qg = psQ.tile([128, CCHUNK * 128], F32, tag="qg")
                            for t in range(c0, c0 + k):
                                nc.tensor.matmul(
                                    out=qg[:, (t - c0) * 128:(t - c0 + 1) * 128],
                                    lhsT=ST[:, (toff + t) * 128:(toff + t + 1) * 128],
                                    rhs=qall[:, blk], start=True, stop=True)
                            nc.vector.tensor_tensor(
                                out=_v(kq[:], (toff + c0) * 128, [[128, k], [1, 128]]),
                                in0=_v(kvg[:], (goff + c0) * 256, [[256, k], [1, 128]]),
                                in1=_v(qg[:], 0, [[128, k], [1, 128]]),
                                op=OP.mult)
                    alpha = sbs.tile([128, Tmax * 4], F32, tag="alpha")
                    nc.vector.tensor_reduce(
                        out=alpha[:, :T * 4],
                        in_=_v(kq[:], 0, [[32, T * 4], [1, 32]]),
                        axis=mybir.AxisListType.X, op=OP.add)
                    r = sb.tile([128, Tmax * 132], F16, tag="r")
                    if EX32:
                        # exp() straight into the 32-wide replicated layout the
                        # v*ex multiply wants -> that multiply runs in 2x mode
                        ex32 = expool.tile([128, Tmax * 128], F16, tag="ex32")
                        nc.scalar.activation(
                            out=_v(ex32[:], 0, [[128, T], [32, 4], [1, 32]]),
                            in_=_v(alpha[:], 0, [[4, T], [1, 4], [0, 32]]),
                            func=AF.Exp)
                        for (kvg, toff, goff, tn) in halves:
                            if tn:
                                nc.vector.tensor_tensor(
                                    out=_v(r[:], toff * 132, [[132, tn], [32, 4], [1, 32]]),
                                    in0=_v(kvg[:], goff * 256 + 128, [[256, tn], [32, 4], [1, 32]]),
                                    in1=_v(ex32[:], toff * 128, [[128, tn], [32, 4], [1, 32]]),
                                    op=OP.mult)
                        nc.scalar.activation(
                            out=_v(r[:], 128, [[132, T], [1, 4]]),
                            in_=_v(alpha[:], 0, [[4, T], [1, 4]]), func=AF.Exp)
                    else:
                        ex = sbs.tile([128, Tmax * 4], F16, tag="ex")
                        nc.scalar.activation(ex[:, :T * 4], alpha[:, :T * 4], AF.Exp)
                        for (kvg, toff, goff, tn) in halves:
                            if tn:
                                nc.vector.tensor_tensor(
                                    out=_v(r[:], toff * 132, [[132, tn], [32, 4], [1, 32]]),
                                    in0=_v(kvg[:], goff * 256 + 128, [[256, tn], [32, 4], [1, 32]]),
                                    in1=_v(ex[:], toff * 4, [[4, tn], [1, 4], [0, 32]]),
                                    op=OP.mult)
                        nc.scalar.activation(
                            out=_v(r[:], 128, [[132, T], [1, 4]]),
                            in_=_v(ex[:], 0, [[4, T], [1, 4]]), func=AF.Identity)
                    agg = psG.tile([128, 132], F32, tag="agg")
                    for t in range(T):
                        nc.tensor.matmul(out=agg[:],
                                         lhsT=S[:, t * 128:(t + 1) * 128],
                                         rhs=r[:, t * 132:(t + 1) * 132],
                                         start=(t == 0), stop=(t == T - 1))
                    rds = sbs.tile([128, 4], F32, tag="rds")
                    nc.vector.tensor_scalar_add(rds[:], agg[:, 128:132], EPS)
                    rd = sbs.tile([128, 4], F32, tag="rd")
                    nc.vector.reciprocal(rd[:], rds[:])
                    nc.vector.tensor_tensor(
                        out=_v(aggn_all[:], b * 128, [[32, 4], [1, 32]]),
                        in0=_v(agg[:], 0, [[32, 4], [1, 32]]),
                        in1=_v(rd[:], 0, [[1, 4], [0, 32]]),
                        op=OP.mult)
            # ---- edge pass B: gelu + output proj + skip ----
            # tanh-approx gelu: 2*gelu(x) ~= x*tanh(GELU_S*(x + GELU_C*x^3)) + x
            # (the 0.5 is folded into Wo). Square and Tanh share activation
            # table set 0 with Exp/Identity -> no table reloads.
            for b in range(BPC):
                blk = slice(b * 128, (b + 1) * 128)
                anT = psB.tile([128, 128], F16, tag="anT")
                nc.tensor.transpose(out=anT[:], in_=aggn_all[:, blk],
                                    identity=ident[:])
                sq = sbs.tile([128, 128], F16, tag="sq")
                nc.scalar.activation(sq[:], anT[:], AF.Square,
                                     scale=math.sqrt(GELU_C))
                m2 = sbs.tile([128, 128], F16, tag="m2")
                nc.vector.tensor_tensor(out=m2[:], in0=sq[:], in1=anT[:],
                                        op=OP.mult)
                nc.vector.tensor_tensor(out=m2[:], in0=m2[:], in1=anT[:],
                                        op=OP.add)
                th = sbs.tile([128, 128], F16, tag="th")
                nc.scalar.activation(th[:], m2[:], AF.Tanh, scale=GELU_S)
                gT = sbs.tile([128, 128], F16, tag="gT")
                nc.vector.tensor_tensor(out=gT[:], in0=th[:], in1=anT[:],
                                        op=OP.mult)
                nc.vector.tensor_tensor(out=gT[:], in0=gT[:], in1=anT[:],
                                        op=OP.add)
                hps = psB.tile([128, 128], F32, tag="hps")
                if l == 1:
                    nc.tensor.matmul(out=hps[:], lhsT=W[f"Wo{l}"][:], rhs=gT[:],
                                     start=True, stop=False)
                    nc.tensor.matmul(out=hps[:], lhsT=skipid[:],
                                     rhs=srcT_of(li, b), start=False, stop=True)
                    nc.scalar.activation(srcT_of(1, b), hps[:], AF.Identity,
                                         bias=W[f"boaT{l}"][:], scale=asig)
                else:
                    nc.tensor.matmul(out=hps[:], lhsT=W[f"Wo{l}"][:], rhs=gT[:],
                                     start=True, stop=True)
                    ha = sbs.tile([128, 128], F16, tag="ha")
                    nc.scalar.activation(ha[:], hps[:], AF.Identity,
                                         bias=W[f"boaT{l}"][:], scale=asig)
                    # uv = w12.T @ (asig*out+bo) + ((1-asig)*w12).T @ h1
                    uvp = psB.tile([2, 128], F32, tag="uvp")
                    nc.tensor.matmul(out=uvp[:], lhsT=W["w12"][:], rhs=ha[:],
                                     start=True, stop=False)
                    nc.tensor.matmul(out=uvp[:], lhsT=W["w12b"][:],
                                     rhs=srcT_of(li, b), start=False, stop=True)
                    uvt = sbs.tile([2, 128], F32, tag="uvt")
                    nc.scalar.activation(uvt[:], uvp[:], AF.Identity)
                    nc.sync.dma_start(uv_out[:, blk], uvt[:])

        layer(0, asig1)
        layer(1, asig2)

    nc.compile()
    return nc


_CACHE = {}


def _get_program(meta, asig1, asig2, blp, gelu_mode=None, nqueues=None):
    if gelu_mode is None:
        gelu_mode = os.environ.get("HGT_GELU", "hw")
    if nqueues is None:
        nqueues = int(os.environ.get("HGT_NQUEUES", "4"))
    key = (meta["N"], meta["E"], meta["P"], meta["T1_b"], meta["T2_b"],
           asig1, asig2, gelu_mode, nqueues)
    if key not in _CACHE:
        _CACHE[key] = _build_program(meta, asig1, asig2, gelu_mode, nqueues)
    return _CACHE[key]


def make_in_maps(inputs):
    inputs = {k: np.asarray(v) for k, v in inputs.items()}
    H, D = inputs["a1"].shape[0], inputs["a1"].shape[1]
    meta, arrays = _host_prep(inputs["x"].astype(np.float32),
                              inputs["edge_index"],
                              inputs["pos_edge_index"],
                              inputs["neg_edge_index"])
    w = _prep_weights(inputs, H, D)
    in_maps = []
    for c in range(CORES):
        m = dict(xT=arrays["xT"][c], kv16=arrays["kv16"][c],
                 S_hot=arrays["S"][c], ST_hot=arrays["ST"][c],
                 ident_in=arrays["ident"])
        for n in ("Wqkv1", "Wqkv2", "bqkv1", "bqkv2", "Wo1", "Wo2",
                  "boaT1", "boaT2", "w12", "w12b"):
            m[n] = w[n]
        in_maps.append(m)
    return meta, w, in_maps


def assemble(meta, results, inputs, blp):
    uv = np.concatenate([results[c]["uvT_out"] for c in range(CORES)], axis=1)
    u1, u2 = uv[0], uv[1]
    pe, ne = inputs["pos_edge_index"], inputs["neg_edge_index"]
    pos = u1[pe[0]] + u2[pe[1]] + np.float32(blp)
    neg = u1[ne[0]] + u2[ne[1]] + np.float32(blp)
    return pos.astype(np.float32), neg.astype(np.float32)


def kernel(**inputs):
    meta, w, in_maps = make_in_maps(inputs)
    nc = _get_program(meta, w["asig1"], w["asig2"], w["blp"])
    res = bass_utils.run_bass_kernel_spmd(nc, in_maps,
                                          core_ids=list(range(CORES)))
    return assemble(meta, res.results, inputs, w["blp"])
